# revision 1
# baseline (speedup 1.0000x reference)
"""Joint soft-histogram kernel for Trainium2 (Bass/Tile), 8-core data parallel.

Math (per batch b, K=256, L=1/256, W=L/2.5, N=65536 pixels):
    phi_k(x) = sigmoid((x - k*L)/W) - sigmoid((x - (k+1)*L)/W)
             = S_k(x) - S_{k+1}(x),   S_k(x) = sigmoid(640*x - 2.5*k)
    out[k, j] = sum_n phi_k(x_n) * phi_j(y_n) / N

Half-telescope: out[k, j] = (H[k, j] - H[k, j+1]) / N with H = Phi_x^T @ T_y,
T_y[n, j] = S_j(y_n), j = 0..256. Only the x side needs the adjacent
difference before the matmul; the y-side difference collapses onto the tiny
(256, 257) H. H entries stay O(256), so fp32 PSUM accumulation is safe, and
fp16 Phi/T operands give ~3e-4 relative error overall.

Pipeline per 16-chunk group (chunk = 128 pixels on partitions):
  - DVE tensor_scalar: A = krow + 640*x_col  (pre-activation, fp32)
  - ScalarE one big-free-dim sigmoid over the group (amortizes the ~224cyc
    per-instruction overhead)
  - DVE (or GPSIMD) adjacent diff -> phi (fp16)
  - TensorE: 2 matmuls per chunk accumulate H in PSUM.
The y side can either be staged the same way (no diff needed) or computed
with one per-chunk activation using the per-partition bias operand.

Sharding: pure data parallel, batch b -> core b.
"""

import numpy as np

import concourse.bass as bass
import concourse.tile as tile
from concourse import bacc, mybir
from concourse.bass_utils import run_bass_kernel_spmd

F32 = mybir.dt.float32
F16 = mybir.dt.float16

B = 8
K = 256
KB = K + 1            # 257 bins on the S/T axis
KP = KB + 1           # 258: padded per-chunk stride; even innermost dim is
                      # required for the DVE tensor_scalar 2x_2P perf mode
NPIX = 65536
NCHUNK = 512
XG = 16               # chunks per staged group
NG = NCHUNK // XG     # 32 groups
GF = XG * KP          # staged group free size (4128)
SCALE = 640.0
INV_N = 1.0 / NPIX

# --- tuning knobs -----------------------------------------------------------
# stage the y side (DVE pre-add + big ACT) for group g, else per-chunk ACT-bias
Y_STAGED = [True] * NG
# engine for the x-side adjacent diff per group: 'v' (vector) or 'g' (gpsimd)
# gpsimd TT measured ~3x slower than DVE; give it ~19/32 groups so
# DVE(pre-adds + 13 diffs) ~ GPSIMD(19 diffs)
DIFF_ENG = ['g' if (g % 5) != 2 else 'v' for g in range(NG)]
DIFF_ENG = ['v' if (g % 3) == 0 else 'g' for g in range(NG)]
# ---------------------------------------------------------------------------

_cached_nc = None


def _build():
    # Bacc (not plain Bass): its finalize() runs generate_event_semaphores,
    # which splits multi-wait instructions that TRN2 engines cannot encode.
    nc = bacc.Bacc("TRN2")
    xd = nc.declare_dram_parameter("x", [128, 512], F32, isOutput=False)
    yd = nc.declare_dram_parameter("y", [128, 512], F32, isOutput=False)
    kd = nc.declare_dram_parameter("krow", [128, KP], F32, isOutput=False)
    od = nc.declare_dram_parameter("out", [256, 256], F32, isOutput=True)

    sig = mybir.ActivationFunctionType.Sigmoid
    add = mybir.AluOpType.add

    with tile.TileContext(nc) as tc:
        with (
            tc.tile_pool(name="singles", bufs=1) as singles,
            tc.tile_pool(name="stage32", bufs=2) as stage32,
            tc.tile_pool(name="stage16", bufs=3) as stage16,
            tc.tile_pool(name="work", bufs=4) as work,
            tc.tile_pool(name="psum", bufs=1, space="PSUM") as psum,
        ):
            xt = singles.tile([128, 512], F32)
            nc.sync.dma_start(out=xt, in_=xd[:, :])
            yt = singles.tile([128, 512], F32)
            nc.sync.dma_start(out=yt, in_=yd[:, :])
            kr = singles.tile([128, KP], F32)
            nc.sync.dma_start(out=kr, in_=kd[:, :])

            x6 = singles.tile([128, 512], F32)
            nc.vector.tensor_scalar_mul(out=x6, in0=xt, scalar1=SCALE)
            y6 = singles.tile([128, 512], F32)
            nc.vector.tensor_scalar_mul(out=y6, in0=yt, scalar1=SCALE)

            H = psum.tile([128, 2, 512], F32)

            for g in range(NG):
                c0 = g * XG
                # ---- x side: staged pre-add + one big sigmoid + group diff
                ax = stage32.tile([128, GF], F32, tag="ax")
                for i in range(XG):
                    nc.vector.tensor_scalar(
                        out=ax[:, i * KP : (i + 1) * KP],
                        in0=kr,
                        scalar1=x6[:, c0 + i : c0 + i + 1],
                        scalar2=None,
                        op0=add,
                    )
                sx = stage16.tile([128, GF], F16, tag="sx")
                nc.scalar.activation(out=sx, in_=ax, func=sig)
                ph = stage16.tile([128, GF], F16, tag="ph")
                if DIFF_ENG[g] == 'g':
                    nc.gpsimd.tensor_sub(
                        out=ph[:, 0 : GF - 1], in0=sx[:, 0 : GF - 1],
                        in1=sx[:, 1:GF],
                    )
                else:
                    nc.vector.tensor_sub(
                        out=ph[:, 0 : GF - 1], in0=sx[:, 0 : GF - 1],
                        in1=sx[:, 1:GF],
                    )

                # ---- y side
                if Y_STAGED[g]:
                    ay = stage32.tile([128, GF], F32, tag="ay")
                    for i in range(XG):
                        nc.vector.tensor_scalar(
                            out=ay[:, i * KP : (i + 1) * KP],
                            in0=kr,
                            scalar1=y6[:, c0 + i : c0 + i + 1],
                            scalar2=None,
                            op0=add,
                        )
                    tyg = stage16.tile([128, GF], F16, tag="tyg")
                    nc.scalar.activation(out=tyg, in_=ay, func=sig)

                # ---- matmuls
                for i in range(XG):
                    c = c0 + i
                    if Y_STAGED[g]:
                        ty = tyg[:, i * KP : i * KP + KB]
                    else:
                        tyt = work.tile([128, KB], F16, tag="ty")
                        nc.scalar.activation(
                            out=tyt, in_=kr[:, 0:KB], func=sig,
                            bias=y6[:, c : c + 1], scale=1.0,
                        )
                        ty = tyt
                    first = c == 0
                    last = c == NCHUNK - 1
                    nc.tensor.matmul(
                        H[:, 0, 0:KB],
                        lhsT=ph[:, i * KP : i * KP + 128],
                        rhs=ty,
                        start=first,
                        stop=last,
                    )
                    nc.tensor.matmul(
                        H[:, 1, 0:KB],
                        lhsT=ph[:, i * KP + 128 : i * KP + 256],
                        rhs=ty,
                        start=first,
                        stop=last,
                    )

            for h in range(2):
                t1 = work.tile([128, KB], F32, tag="ep")
                nc.scalar.activation(
                    out=t1,
                    in_=H[:, h, 0:KB],
                    func=mybir.ActivationFunctionType.Copy,
                    scale=INV_N,
                )
                t2 = work.tile([128, K], F32, tag="ep2")
                nc.vector.tensor_sub(out=t2, in0=t1[:, 0:K], in1=t1[:, 1:KB])
                nc.sync.dma_start(out=od[128 * h : 128 * (h + 1), :], in_=t2)

    nc.finalize()
    return nc


def _get_nc():
    global _cached_nc
    if _cached_nc is None:
        _cached_nc = _build()
    return _cached_nc


def _krow():
    row = np.arange(KP, dtype=np.float32) * np.float32(-2.5)
    return np.tile(row[None, :], (128, 1))


def _in_maps(x, y):
    x = np.ascontiguousarray(np.asarray(x, dtype=np.float32))
    y = np.ascontiguousarray(np.asarray(y, dtype=np.float32))
    kr = _krow()
    return [
        {
            "x": x[b].reshape(128, 512),
            "y": y[b].reshape(128, 512),
            "krow": kr,
        }
        for b in range(B)
    ]


def run(x, y, trace=False, **trace_kw):
    """Run on all 8 cores; returns (out (8,256,256) f32, BassKernelResults)."""
    nc = _get_nc()
    res = run_bass_kernel_spmd(nc, _in_maps(x, y), list(range(B)), trace=trace,
                               **trace_kw)
    out = np.stack([res.results[b]["out"] for b in range(B)]).astype(np.float32)
    return out, res


def kernel(x, y):
    out, _ = run(x, y)
    return out



# revision 2
# speedup vs baseline: 1.6365x; 1.6365x over previous
"""Joint soft-histogram kernel for Trainium2 (Bass/Tile), 8-core data parallel.

Math (per batch b, K=256, L=1/256, W=L/2.5, N=65536 pixels):
    phi_k(x) = S_k(x) - S_{k+1}(x),   S_k(x) = sigmoid(640*x - 2.5*k)
    out[k, j] = sum_n phi_k(x_n) * phi_j(y_n) / N

Half-telescope on y: out[k, j] = (H[k, j] - H[k, j+1]) / N with
H[k, j] = sum_n phi_k(x_n) * S_j(y_n).  H entries are O(256), so fp32 PSUM
accumulation is safe (the x-side diff must stay pre-matmul: a double
telescope would accumulate O(65536) values and lose ~0.1 absolute to
roundoff).

The x-side diff is folded into the matmul instead of a DVE tensor_sub:
    H[k,:] += Sx[:,k]^T @ Sy   and   H[k,:] += (-Sx[:,k+1])^T @ Sy
using a negated copy of the staged sigmoid tile as the shifted lhsT
(one 4x-mode DVE negation per 16-chunk group).

Per-chunk pre-adds (krow + 640*x_c) write fp16 from an fp16 krow tile:
tensor_scalar with a per-partition fp32 scalar AP runs in 2x_1P mode when
the tensor operands are 16-bit step-1 (measured 275ns vs 470ns for fp32).
fp16 argument rounding only perturbs sigmoid args by <=2^-8 where
non-saturated (rel-err impact ~1e-3, tolerance is 2e-2).

A fraction of the y-side chunks skip the pre-add entirely and use a
per-chunk ScalarE activation with per-partition bias (dur ~510ns),
balancing the DVE and ScalarE loads.

GPSIMD is not used at all: measured ~14.5ns/col for tensor ops AND it
stalls concurrent DVE ops via the shared SBUF port.

Sharding: pure data parallel, batch b -> core b.
"""

import numpy as np

import concourse.bass as bass
import concourse.tile as tile
from concourse import bacc, mybir
from concourse.bass_utils import run_bass_kernel_spmd

F32 = mybir.dt.float32
F16 = mybir.dt.float16

B = 8
K = 256
KB = 258              # sigmoid columns per chunk (j = 0..257; even for 2x DVE)
NPIX = 65536
NCHUNK = 512
XG = 16               # chunks per staged group
NG = NCHUNK // XG     # 32 groups
NBIAS = 4             # per group: y-side chunks using bias-act (no pre-add)
NSTAGE = XG - NBIAS   # y-side chunks staged via DVE pre-add
SCALE = 640.0
INV_N = 1.0 / NPIX

sig = mybir.ActivationFunctionType.Sigmoid
add = mybir.AluOpType.add

_cached_nc = None


def _build():
    nc = bacc.Bacc("TRN2")
    xd = nc.declare_dram_parameter("x", [128, 512], F32, isOutput=False)
    yd = nc.declare_dram_parameter("y", [128, 512], F32, isOutput=False)
    kd = nc.declare_dram_parameter("krow", [128, KB], F16, isOutput=False)
    od = nc.declare_dram_parameter("out", [256, 256], F32, isOutput=True)

    GFX = XG * KB         # x-side staged free size (4128)
    GFY = NSTAGE * KB     # y-side staged free size

    with tile.TileContext(nc) as tc:
        with (
            tc.tile_pool(name="singles", bufs=1) as singles,
            tc.tile_pool(name="stage", bufs=2) as stage,
            tc.tile_pool(name="work", bufs=2) as work,
            tc.tile_pool(name="psum", bufs=1, space="PSUM") as psum,
        ):
            xt = singles.tile([128, 512], F32)
            nc.sync.dma_start(out=xt, in_=xd[:, :])
            yt = singles.tile([128, 512], F32)
            nc.sync.dma_start(out=yt, in_=yd[:, :])
            kr = singles.tile([128, KB], F16)
            nc.sync.dma_start(out=kr, in_=kd[:, :])

            x6 = singles.tile([128, 512], F32)
            nc.vector.tensor_scalar_mul(out=x6, in0=xt, scalar1=SCALE)
            y6 = singles.tile([128, 512], F32)
            nc.vector.tensor_scalar_mul(out=y6, in0=yt, scalar1=SCALE)

            H = psum.tile([128, 2, 512], F32)

            for g in range(NG):
                c0 = g * XG

                # ---- x side: 16 fp16 pre-adds + one big sigmoid + negate
                ax = stage.tile([128, GFX], F16, tag="ax")
                for i in range(XG):
                    nc.vector.tensor_scalar(
                        out=ax[:, i * KB : (i + 1) * KB],
                        in0=kr,
                        scalar1=x6[:, c0 + i : c0 + i + 1],
                        scalar2=None,
                        op0=add,
                    )
                sx = stage.tile([128, GFX], F16, tag="sx")
                nc.scalar.activation(out=sx, in_=ax, func=sig)
                sxn = stage.tile([128, GFX], F16, tag="sxn")
                nc.vector.tensor_scalar_mul(out=sxn, in0=sx, scalar1=-1.0)

                # ---- y side: NSTAGE staged pre-adds + big act; NBIAS bias-acts
                sy = stage.tile([128, GFX], F16, tag="sy")
                ay = stage.tile([128, GFY], F16, tag="ay")
                for i in range(NSTAGE):
                    nc.vector.tensor_scalar(
                        out=ay[:, i * KB : (i + 1) * KB],
                        in0=kr,
                        scalar1=y6[:, c0 + i : c0 + i + 1],
                        scalar2=None,
                        op0=add,
                    )
                nc.scalar.activation(out=sy[:, 0:GFY], in_=ay, func=sig)
                for i in range(NSTAGE, XG):
                    nc.scalar.activation(
                        out=sy[:, i * KB : (i + 1) * KB],
                        in_=kr,
                        func=sig,
                        bias=y6[:, c0 + i : c0 + i + 1],
                        scale=1.0,
                    )

                # ---- matmuls: 4 per chunk (x-diff via shifted negated lhsT)
                for i in range(XG):
                    c = c0 + i
                    first = c == 0
                    last = c == NCHUNK - 1
                    o = i * KB
                    rhs = sy[:, o : o + KB]
                    nc.tensor.matmul(
                        H[:, 0, 0:KB],
                        lhsT=sx[:, o : o + 128],
                        rhs=rhs,
                        start=first,
                        stop=False,
                    )
                    nc.tensor.matmul(
                        H[:, 0, 0:KB],
                        lhsT=sxn[:, o + 1 : o + 129],
                        rhs=rhs,
                        start=False,
                        stop=last,
                    )
                    nc.tensor.matmul(
                        H[:, 1, 0:KB],
                        lhsT=sx[:, o + 128 : o + 256],
                        rhs=rhs,
                        start=first,
                        stop=False,
                    )
                    nc.tensor.matmul(
                        H[:, 1, 0:KB],
                        lhsT=sxn[:, o + 129 : o + 257],
                        rhs=rhs,
                        start=False,
                        stop=last,
                    )

            # ---- endgame: scale, y-diff, store
            for h in range(2):
                t1 = work.tile([128, KB], F32, tag="ep")
                nc.scalar.activation(
                    out=t1,
                    in_=H[:, h, 0:KB],
                    func=mybir.ActivationFunctionType.Copy,
                    scale=INV_N,
                )
                t2 = work.tile([128, K], F32, tag="ep2")
                nc.vector.tensor_sub(out=t2, in0=t1[:, 0:K], in1=t1[:, 1 : K + 1])
                nc.sync.dma_start(out=od[128 * h : 128 * (h + 1), :], in_=t2)

    nc.finalize()
    return nc


def _get_nc():
    global _cached_nc
    if _cached_nc is None:
        _cached_nc = _build()
    return _cached_nc


def _krow():
    row = (np.arange(KB, dtype=np.float32) * np.float32(-2.5)).astype(np.float16)
    return np.tile(row[None, :], (128, 1))


def _in_maps(x, y):
    x = np.ascontiguousarray(np.asarray(x, dtype=np.float32))
    y = np.ascontiguousarray(np.asarray(y, dtype=np.float32))
    kr = _krow()
    return [
        {
            "x": x[b].reshape(128, 512),
            "y": y[b].reshape(128, 512),
            "krow": kr,
        }
        for b in range(B)
    ]


def run(x, y, trace=False, **trace_kw):
    """Run on all 8 cores; returns (out (8,256,256) f32, BassKernelResults)."""
    nc = _get_nc()
    res = run_bass_kernel_spmd(nc, _in_maps(x, y), list(range(B)), trace=trace,
                               **trace_kw)
    out = np.stack([res.results[b]["out"] for b in range(B)]).astype(np.float32)
    return out, res


def kernel(x, y):
    out, _ = run(x, y)
    return out


# revision 3
# speedup vs baseline: 1.6666x; 1.0184x over previous
"""Joint soft-histogram kernel for Trainium2 (Bass/Tile), 8-core data parallel.

Math (per batch b, K=256, L=1/256, W=L/2.5, N=65536 pixels):
    phi_k(x) = S_k(x) - S_{k+1}(x),   S_k(x) = sigmoid(640*x - 2.5*k)
    out[k, j] = sum_n phi_k(x_n) * phi_j(y_n) / N

Half-telescope on y: out[k, j] = (H[k, j] - H[k, j+1]) / N with
H[k, j] = sum_n phi_k(x_n) * S_j(y_n).  H entries are O(256), so fp32 PSUM
accumulation is safe (a double telescope would accumulate O(65536) values
and lose ~0.1 absolute to roundoff -> 10% error; the x-side diff must stay
pre-matmul).

The x-side diff is folded into the matmul instead of a DVE tensor_sub:
    H[k,:] += Sx[:,k]^T @ Sy   and   H[k,:] += (-Sx[:,k+1])^T @ Sy
using a negated copy of the staged sigmoid tile as the shifted lhsT
(one 4x-mode DVE negation per 16-chunk group).

Per-chunk pre-adds (krow + 640*x_c) write fp16 from an fp16 krow tile:
tensor_scalar with a per-partition fp32 scalar AP runs in 4x mode when the
tensor operands are 16-bit step-1 (measured ~287ns at FD=258, dominated by
the ~220-cycle instruction init).  fp16 argument rounding only perturbs
sigmoid args by <=2^-8 where non-saturated (rel-err impact ~1e-3 against a
2e-2 tolerance).

A few y-side chunks per group (NBIAS, placed FIRST so their sigmoids are
ready before the matmul stream reaches them) skip the pre-add and use a
per-chunk ScalarE activation with per-partition bias (~510ns), balancing
DVE vs ScalarE load.  All staged pre-adds (x and y) share ONE activation
instruction per group to amortize the ~290ns ScalarE per-instruction
overhead.

GPSIMD is not used at all: measured ~14.5ns/col for tensor ops AND it
stalls concurrent DVE ops via the shared SBUF port.

Sharding: pure data parallel, batch b -> core b.
"""

import numpy as np

import concourse.bass as bass
import concourse.tile as tile
from concourse import bacc, mybir
from concourse.bass_utils import run_bass_kernel_spmd

F32 = mybir.dt.float32
F16 = mybir.dt.float16

B = 8
K = 256
KB = 258              # sigmoid columns per chunk (j = 0..257; even for DVE modes)
NPIX = 65536
NCHUNK = 512
XG = 16               # chunks per staged group
NG = NCHUNK // XG     # 32 groups
NBIAS = 3             # per group: y-side chunks using bias-act (no pre-add)
SCALE = 640.0
INV_N = 1.0 / NPIX

sig = mybir.ActivationFunctionType.Sigmoid
add = mybir.AluOpType.add

_cached_nc = None


def _build():
    nc = bacc.Bacc("TRN2")
    xd = nc.declare_dram_parameter("x", [128, 512], F32, isOutput=False)
    yd = nc.declare_dram_parameter("y", [128, 512], F32, isOutput=False)
    kd = nc.declare_dram_parameter("krow", [128, KB], F16, isOutput=False)
    od = nc.declare_dram_parameter("out", [256, 256], F32, isOutput=True)

    NSTAGE = XG - NBIAS   # y-side chunks staged via DVE pre-add
    GFX = XG * KB         # x-side staged free size (4128)
    GFY = NSTAGE * KB     # y-side staged free size
    GF = GFX + GFY        # merged stage width

    with tile.TileContext(nc) as tc:
        with (
            tc.tile_pool(name="singles", bufs=1) as singles,
            tc.tile_pool(name="stage", bufs=2) as stage,
            tc.tile_pool(name="work", bufs=2) as work,
            tc.tile_pool(name="psum", bufs=1, space="PSUM") as psum,
        ):
            xt = singles.tile([128, 512], F32)
            nc.sync.dma_start(out=xt, in_=xd[:, :])
            yt = singles.tile([128, 512], F32)
            nc.sync.dma_start(out=yt, in_=yd[:, :])
            kr = singles.tile([128, KB], F16)
            nc.sync.dma_start(out=kr, in_=kd[:, :])

            x6 = singles.tile([128, 512], F32)
            nc.vector.tensor_scalar_mul(out=x6, in0=xt, scalar1=SCALE)
            y6 = singles.tile([128, 512], F32)
            nc.vector.tensor_scalar_mul(out=y6, in0=yt, scalar1=SCALE)

            H = psum.tile([128, 2, 512], F32)

            for g in range(NG):
                c0 = g * XG

                # ---- bias-act y chunks (first NBIAS of the group): ScalarE
                # computes these while the DVE fills the staged pre-adds.
                syb = stage.tile([128, NBIAS * KB], F16, tag="syb")
                for i in range(NBIAS):
                    nc.scalar.activation(
                        out=syb[:, i * KB : (i + 1) * KB],
                        in_=kr,
                        func=sig,
                        bias=y6[:, c0 + i : c0 + i + 1],
                        scale=1.0,
                    )

                # ---- staged pre-adds: x chunks 0..15, y chunks NBIAS..15,
                # all into one tile -> ONE group sigmoid.
                axy = stage.tile([128, GF], F16, tag="axy")
                for i in range(XG):
                    nc.vector.tensor_scalar(
                        out=axy[:, i * KB : (i + 1) * KB],
                        in0=kr,
                        scalar1=x6[:, c0 + i : c0 + i + 1],
                        scalar2=None,
                        op0=add,
                    )
                for i in range(NBIAS, XG):
                    o = GFX + (i - NBIAS) * KB
                    nc.vector.tensor_scalar(
                        out=axy[:, o : o + KB],
                        in0=kr,
                        scalar1=y6[:, c0 + i : c0 + i + 1],
                        scalar2=None,
                        op0=add,
                    )
                sxy = stage.tile([128, GF], F16, tag="sxy")
                nc.scalar.activation(out=sxy, in_=axy, func=sig)
                sxn = stage.tile([128, GFX], F16, tag="sxn")
                nc.vector.tensor_scalar_mul(
                    out=sxn, in0=sxy[:, 0:GFX], scalar1=-1.0)

                # ---- matmuls: 4 per chunk (x-diff via shifted negated lhsT)
                for i in range(XG):
                    c = c0 + i
                    first = c == 0
                    last = c == NCHUNK - 1
                    o = i * KB
                    if i < NBIAS:
                        rhs = syb[:, i * KB : i * KB + KB]
                    else:
                        oy = GFX + (i - NBIAS) * KB
                        rhs = sxy[:, oy : oy + KB]
                    nc.tensor.matmul(
                        H[:, 0, 0:KB],
                        lhsT=sxy[:, o : o + 128],
                        rhs=rhs,
                        start=first,
                        stop=False,
                    )
                    nc.tensor.matmul(
                        H[:, 0, 0:KB],
                        lhsT=sxn[:, o + 1 : o + 129],
                        rhs=rhs,
                        start=False,
                        stop=last,
                    )
                    nc.tensor.matmul(
                        H[:, 1, 0:KB],
                        lhsT=sxy[:, o + 128 : o + 256],
                        rhs=rhs,
                        start=first,
                        stop=False,
                    )
                    nc.tensor.matmul(
                        H[:, 1, 0:KB],
                        lhsT=sxn[:, o + 129 : o + 257],
                        rhs=rhs,
                        start=False,
                        stop=last,
                    )

            # ---- endgame: scale, y-diff, store
            for h in range(2):
                t1 = work.tile([128, KB], F32, tag="ep")
                nc.scalar.activation(
                    out=t1,
                    in_=H[:, h, 0:KB],
                    func=mybir.ActivationFunctionType.Copy,
                    scale=INV_N,
                )
                t2 = work.tile([128, K], F32, tag="ep2")
                nc.vector.tensor_sub(out=t2, in0=t1[:, 0:K], in1=t1[:, 1 : K + 1])
                nc.sync.dma_start(out=od[128 * h : 128 * (h + 1), :], in_=t2)

    nc.finalize()
    return nc


def _get_nc():
    global _cached_nc
    if _cached_nc is None:
        _cached_nc = _build()
    return _cached_nc


def _krow():
    row = (np.arange(KB, dtype=np.float32) * np.float32(-2.5)).astype(np.float16)
    return np.tile(row[None, :], (128, 1))


def _in_maps(x, y):
    x = np.ascontiguousarray(np.asarray(x, dtype=np.float32))
    y = np.ascontiguousarray(np.asarray(y, dtype=np.float32))
    kr = _krow()
    return [
        {
            "x": x[b].reshape(128, 512),
            "y": y[b].reshape(128, 512),
            "krow": kr,
        }
        for b in range(B)
    ]


def run(x, y, trace=False, **trace_kw):
    """Run on all 8 cores; returns (out (8,256,256) f32, BassKernelResults)."""
    nc = _get_nc()
    res = run_bass_kernel_spmd(nc, _in_maps(x, y), list(range(B)), trace=trace,
                               **trace_kw)
    out = np.stack([res.results[b]["out"] for b in range(B)]).astype(np.float32)
    return out, res


def kernel(x, y):
    out, _ = run(x, y)
    return out
